# revision 15
# baseline (speedup 1.0000x reference)
"""Trainium2 Bass kernel for nn_CausalSE: causal cumulative-average pooling
+ squeeze-excite gating, data-parallel over batch (one NeuronCore per batch
element).

Reference math per batch element (D=512, T=8192, chunk=16, Tc=512):
    avg    = cumsum(x, t) / (t+1)
    pooled = avg[:, 15::16]                          # [D, Tc]
    h      = relu(w1 @ pooled + b1)                  # [64, Tc]
    g      = sigmoid(w2 @ h + b2)                    # [D, Tc]
    out    = repeat(g, 16, t)[:, :T] * x

The kernel is HBM-bound: per core it streams x in and out once.  x
crosses HBM as fp16 (host converts both ways): ~16.8 MB at the ~400
GB/s per-core aggregate DMA rate => ~42us floor + ~10us of fixed NEFF
preamble/postamble, so every compute engine must stay under ~40us.

v2+ restructure (v1 was DVE-bound at ~55us busy):
  - The chunk sums moved off DVE onto the (otherwise idle) PE.
    Because w1 @ chunk_sum(x) == chunk_sum(w1 @ x), the PE computes
    Y = w1 @ x from the resident fp16 x tiles (4 ki accumulation
    steps into PSUM [64, 512] banks) and DVE windowed-reduces the
    8x-smaller Y.  DVE busy drops to ~35us (gate multiplies ~19us +
    reduces ~11us + scan/scale/bias-relu smalls).
  - relu rides DVE as tensor_scalar(add b1, max 0) so ACT needs only
    the sigmoid table set; a dummy 1-element sigmoid up front pulls
    the walrus ACT_TABLE_LOAD into the startup DMA window.
  - DMA: loads are consolidated (one DMA per t-block carrying all 4
    d-tiles) and ordered for the critical path: block-0 + the weights
    the block-0 chain needs go FIRST on the SP ring (empirically the
    faster ring), with remaining loads alternating rings.  w1/b2 are
    host-pre-swizzled to partition-major so their descriptors are
    contiguous (the naive (d p)->p d rearrange makes 128-byte
    descriptors that crawl); b1 is packed into column 0 of the scale
    tensor.  Stores: d01 pairs on SP, d23 pairs on ACT, per-d-tile on
    both rings for the tail block.  GpSimd issues nothing.
  - Engine layout per t-block: PE runs the Y-matmuls ki-outer
    (stationary reuse, PSUM-bank-group interleave) then 4 gate
    matmuls; DVE runs reduces, the causal scan, scale, bias+relu and
    the 4 gate multiplies (deferred one block so DVE never waits on
    ACT); ACT runs the fused sigmoid + 16x upsample (broadcast PSUM
    read, dense fp16 gate out).
"""

import sys

for _p in ("/opt/trn_rl_repo",):
    if _p not in sys.path:
        sys.path.insert(0, _p)

import numpy as np

B, D, T = 8, 512, 8192
DH = 64          # bottleneck dim = D // 8
CS = 16          # chunksize
TC = T // CS     # 512 chunks
NCORES = 8
NDT = D // 128   # 4 partition tiles of x / out
SB = 512         # Y-matmul sub-block (one PSUM bank of fp32)
CSB = SB // CS   # 32 chunk columns per sub-block
TBLOCKS = [(0, 512), (512, 1536), (2048, 2048), (4096, 2048),
           (6144, 1536), (7680, 512)]
TBMAX = 2048

_compiled_nc = None


def build_nc():
    import concourse.tile as tile
    from concourse import bacc, mybir

    f32 = mybir.dt.float32
    f16 = mybir.dt.float16
    AF = mybir.ActivationFunctionType
    ALU = mybir.AluOpType
    AX = mybir.AxisListType

    # Bacc (not plain Bass): its finalize() runs the TRN2 sync-wait
    # legalization (move_matmul_waits_to_ldweights / event-semaphore
    # splitting) that walrus codegen requires.
    nc = bacc.Bacc("TRN2", target_bir_lowering=False)
    x_d = nc.declare_dram_parameter("x", [D, T], f16, isOutput=False)
    w1p_d = nc.declare_dram_parameter("w1p", [128, NDT * DH], f16,
                                      isOutput=False)
    w2t_d = nc.declare_dram_parameter("w2t", [DH, D], f16, isOutput=False)
    b2p_d = nc.declare_dram_parameter("b2p", [128, NDT], f32, isOutput=False)
    sclb_d = nc.declare_dram_parameter("sclb", [DH, TC + 1], f32,
                                       isOutput=False)
    out_d = nc.declare_dram_parameter("out", [D, T], f16, isOutput=True)

    with tile.TileContext(nc) as tc:
        with (
            tc.tile_pool(name="xres", bufs=1) as xres,
            tc.tile_pool(name="small", bufs=1) as small,
            tc.tile_pool(name="ups", bufs=2) as ups,
            tc.tile_pool(name="psum_y", bufs=1, space="PSUM") as psum_y,
            tc.tile_pool(name="psum_g", bufs=6, space="PSUM") as psum_g,
        ):
            # x resident in SBUF: [128, 4, 8192] fp16 = 8 MB
            xt = xres.tile([128, NDT, T], f16, tag="x", name="x")
            w1s = small.tile([128, NDT, DH], f16, tag="w1")
            w2s = small.tile([DH, D], f16, tag="w2")
            b2s = small.tile([128, NDT], f32, tag="b2")
            sclb = small.tile([DH, TC + 1], f32, tag="sclb")
            b1s = sclb[:, 0:1]
            scl = sclb[:, 1:TC + 1]
            qs = small.tile([DH, TC], f32, tag="qs")    # causal prefix
            h32 = small.tile([DH, TC], f32, tag="h32")
            h16 = small.tile([DH, TC], f16, tag="h16")
            # per-chunk w1@x sums, accumulated IN PSUM by the Y-matmuls
            # through a stride-0 broadcast output AP (the 16 writes per
            # chunk column hit the same address; has_written makes them
            # accumulate), so no reduce instructions are needed at all
            yq = psum_y.tile([DH, TC], f32, tag="yq", name="yq")

            # Dummy 1-element sigmoid: forces the walrus-inserted
            # ACT_TABLE_LOAD for the sigmoid set to run during the startup
            # DMA window instead of stalling ACT before the first real
            # sigmoid mid-stream.
            dummy = small.tile([1, 2], f32, tag="dummy")
            nc.gpsimd.memset(dummy[:], 0.0)
            nc.scalar.activation(dummy[:, 1:2], dummy[:, 0:1], AF.Sigmoid)

            def load_block(eng, t0, TB):
                eng.dma_start(
                    xt[:, :, t0:t0 + TB],
                    x_d[:, t0:t0 + TB].rearrange("(k p) t -> p k t", p=128),
                )

            # SP ring (fast): the block-0 critical path first (tiny weights,
            # then the b0 x-block), then its share of the loads; ALL the
            # steady-state stores are issued on this ring later so the ACT
            # engine never spends mid-kernel time on DMA issue.
            nc.sync.dma_start(
                w1s[:], w1p_d[:].rearrange("p (d h) -> p d h", d=NDT)
            )
            nc.sync.dma_start(sclb[:], sclb_d[:])
            for bi in (0, 1, 3, 5):
                load_block(nc.sync, *TBLOCKS[bi])
            # ACT ring: the gate-path weights + two mid loads, all issued
            # up front before the first sigmoid.
            nc.scalar.dma_start(w2s[:], w2t_d[:])
            nc.scalar.dma_start(b2s[:], b2p_d[:])
            for bi in (2, 4):
                load_block(nc.scalar, *TBLOCKS[bi])

            # Causal pipeline: gate for chunk c needs only x[:, :16(c+1)].
            # Block k's gate multiplies + stores are emitted AFTER block
            # k+1's reduce/scan stage so the in-order DVE queue never stalls
            # waiting for the ACT sigmoid-upsample (software pipelining).
            deferred = None

            def emit_mults(items, tail=False):
                for di, t0_, TB_, u_ in items:
                    xv = xt[:, di, t0_:t0_ + TB_]
                    nc.vector.tensor_tensor(xv, xv, u_[:, :TB_], op=ALU.mult)
                    if tail:
                        # tail: per-d-tile stores, alternating rings (the
                        # ACT ring is idle by now), right behind each
                        # multiply so the drain pipelines
                        deng = nc.sync if di < 2 else nc.scalar
                        deng.dma_start(
                            out_d[di * 128:(di + 1) * 128, t0_:t0_ + TB_], xv
                        )
                if not tail:
                    di, t0_, TB_, _ = items[0]
                    for half in range(2):
                        nc.sync.dma_start(
                            out_d[half * 256:(half + 1) * 256,
                                  t0_:t0_ + TB_].rearrange(
                                      "(k p) t -> p k t", p=128),
                            xt[:, 2 * half:2 * half + 2, t0_:t0_ + TB_],
                        )

            for tb, (t0, TB) in enumerate(TBLOCKS):
                CB = TB // CS
                c0 = t0 // CS
                nsb = TB // SB
                # q = chunk_sum(w1 @ x) for this block, entirely on PE:
                # ki-outer so each stationary w1-slice is reused across the
                # sub-blocks; the broadcast output AP folds the 16-column
                # chunk sum into the PSUM accumulation itself.
                for ki in range(NDT):
                    for sb in range(nsb):
                        ts = t0 + sb * SB
                        cc = c0 + sb * CSB
                        nc.tensor.matmul(
                            yq[:, cc:cc + CSB].unsqueeze(1).broadcast_to(
                                [DH, CS, CSB]),
                            w1s[:, ki, :],
                            xt[:, ki, ts:ts + SB].rearrange(
                                "p (c j) -> p j c", j=CS),
                            start=(ki == 0),
                            stop=(ki == NDT - 1),
                        )
                # running causal prefix over this block (carry = last col)
                nc.vector.tensor_tensor_scan(
                    qs[:, c0:c0 + CB],
                    yq[:, c0:c0 + CB],
                    scl[:, c0:c0 + CB],
                    0.0 if tb == 0 else qs[:, c0 - 1:c0],
                    op0=ALU.add,
                    op1=ALU.bypass,
                )
                # h = relu(prefix * 1/(16(c+1)) + b1), relu on DVE so ACT
                # only ever needs the sigmoid table set
                nc.vector.tensor_mul(
                    h32[:, c0:c0 + CB], qs[:, c0:c0 + CB], scl[:, c0:c0 + CB]
                )
                nc.vector.tensor_scalar(
                    h16[:, c0:c0 + CB], h32[:, c0:c0 + CB],
                    b1s, 0.0, op0=ALU.add, op1=ALU.max,
                )
                last = tb == len(TBLOCKS) - 1
                if last and deferred is not None:
                    # flush the previous block's multiplies first so the
                    # tail drains in order
                    emit_mults(deferred)
                    deferred = None
                cur = []
                for di in range(NDT):
                    gp = psum_g.tile([128, TBMAX // CS], f32, tag="g",
                                     name="gp")
                    nc.tensor.matmul(
                        gp[:, :CB],
                        w2s[:, di * 128:(di + 1) * 128],
                        h16[:, c0:c0 + CB],
                        start=True,
                        stop=True,
                    )
                    # fused sigmoid + 16x upsample: broadcast-read the
                    # PSUM column per chunk, write the dense fp16 gate
                    u = ups.tile(
                        [128, TBMAX], f16, tag=f"u{di}", name=f"u{di}"
                    )
                    nc.scalar.activation(
                        u[:, :TB].rearrange("p (c j) -> p c j", j=CS),
                        gp[:, :CB].unsqueeze(2).broadcast_to([128, CB, CS]),
                        AF.Sigmoid,
                        bias=b2s[:, di:di + 1],
                    )
                    if last:
                        # tail block: multiply right behind each sigmoid so
                        # the drain pipelines at d-tile granularity
                        emit_mults([(di, t0, TB, u)], tail=True)
                    else:
                        cur.append((di, t0, TB, u))
                if deferred is not None:
                    emit_mults(deferred)
                deferred = cur if not last else None
    # run_bass_via_pjrt serializes nc.m as-is; Bacc defers register
    # allocation and TRN2 sync-wait legalization to finalize(), so it must
    # run here or walrus rejects the BIR.
    nc.finalize()
    return nc


def _host_inputs(x, w1, b1, w2, b2, chunksize):
    x = np.asarray(x)
    w1 = np.asarray(w1, dtype=np.float32)
    b1 = np.ascontiguousarray(np.asarray(b1, dtype=np.float32))
    w2 = np.asarray(w2, dtype=np.float32)
    b2 = np.asarray(b2, dtype=np.float32)
    cs = int(chunksize)
    assert cs == CS and x.shape == (B, D, T), (cs, x.shape)
    x16 = np.ascontiguousarray(x.astype(np.float16))
    # w1 pre-swizzled partition-major: w1p[p, k*DH+h] = w1[h, k*128+p]
    w1p = np.ascontiguousarray(
        w1.T.astype(np.float16).reshape(NDT, 128, DH)
        .transpose(1, 0, 2).reshape(128, NDT * DH)
    )
    w2t = np.ascontiguousarray(w2.T.astype(np.float16))      # [DH, D]
    b2p = np.ascontiguousarray(b2.reshape(NDT, 128).T)       # [128, NDT]
    scale = 1.0 / (CS * np.arange(1, TC + 1, dtype=np.float32))
    sclb = np.ascontiguousarray(np.concatenate(
        [np.broadcast_to(b1[:, None], (DH, 1)),
         np.broadcast_to(scale, (DH, TC))], axis=1,
    ))
    shared = dict(w1p=w1p, w2t=w2t, b2p=b2p, sclb=sclb)
    return x16, shared


def kernel(x, w1, b1, w2, b2, chunksize):
    global _compiled_nc
    from concourse.bass_utils import run_bass_kernel_spmd

    x16, shared = _host_inputs(x, w1, b1, w2, b2, chunksize)
    if _compiled_nc is None:
        _compiled_nc = build_nc()
    in_maps = [
        {"x": np.ascontiguousarray(x16[i]), **shared} for i in range(NCORES)
    ]
    res = run_bass_kernel_spmd(_compiled_nc, in_maps, list(range(NCORES)))
    out = np.stack(
        [res.results[i]["out"] for i in range(NCORES)], axis=0
    ).astype(np.float32)
    return out


# revision 18
# speedup vs baseline: 1.6433x; 1.6433x over previous
"""Trainium2 Bass kernel for nn_CausalSE: causal cumulative-average pooling
+ squeeze-excite gating, data-parallel over batch (one NeuronCore per batch
element).

Reference math per batch element (D=512, T=8192, chunk=16, Tc=512):
    avg    = cumsum(x, t) / (t+1)
    pooled = avg[:, 15::16]                          # [D, Tc]
    h      = relu(w1 @ pooled + b1)                  # [64, Tc]
    g      = sigmoid(w2 @ h + b2)                    # [D, Tc]
    out    = repeat(g, 16, t)[:, :T] * x

The kernel is HBM-bound: per core it streams x in and out once.  x
crosses HBM as fp16 (host converts both ways): ~16.8 MB at the ~400
GB/s per-core aggregate DMA rate => ~42us floor + ~10us of fixed NEFF
preamble/postamble, so every compute engine must stay under ~40us.

v2+ restructure (v1 was DVE-bound at ~55us busy):
  - The chunk sums moved off DVE onto the (otherwise idle) PE.
    Because w1 @ chunk_sum(x) == chunk_sum(w1 @ x), the PE computes
    Y = w1 @ x from the resident fp16 x tiles (4 ki accumulation
    steps into PSUM [64, 512] banks) and DVE windowed-reduces the
    8x-smaller Y.  DVE busy drops to ~35us (gate multiplies ~19us +
    reduces ~11us + scan/scale/bias-relu smalls).
  - relu rides DVE as tensor_scalar(add b1, max 0) so ACT needs only
    the sigmoid table set; a dummy 1-element sigmoid up front pulls
    the walrus ACT_TABLE_LOAD into the startup DMA window.
  - DMA: loads are consolidated (one DMA per t-block carrying all 4
    d-tiles) and ordered for the critical path: block-0 + the weights
    the block-0 chain needs go FIRST on the SP ring (empirically the
    faster ring), with remaining loads alternating rings.  w1/b2 are
    host-pre-swizzled to partition-major so their descriptors are
    contiguous (the naive (d p)->p d rearrange makes 128-byte
    descriptors that crawl); b1 is packed into column 0 of the scale
    tensor.  Stores: d01 pairs on SP, d23 pairs on ACT, per-d-tile on
    both rings for the tail block.  GpSimd issues nothing.
  - Engine layout per t-block: PE runs the Y-matmuls ki-outer
    (stationary reuse, PSUM-bank-group interleave) then 4 gate
    matmuls; DVE runs reduces, the causal scan, scale, bias+relu and
    the 4 gate multiplies (deferred one block so DVE never waits on
    ACT); ACT runs the fused sigmoid + 16x upsample (broadcast PSUM
    read, dense fp16 gate out).
"""

import sys

for _p in ("/opt/trn_rl_repo",):
    if _p not in sys.path:
        sys.path.insert(0, _p)

import numpy as np

B, D, T = 8, 512, 8192
DH = 64          # bottleneck dim = D // 8
CS = 16          # chunksize
TC = T // CS     # 512 chunks
NCORES = 8
NDT = D // 128   # 4 partition tiles of x / out
SB = 512         # Y-matmul sub-block (one PSUM bank of fp32)
CSB = SB // CS   # 32 chunk columns per sub-block
TBLOCKS = [(0, 512), (512, 1536), (2048, 2048), (4096, 2048),
           (6144, 1536), (7680, 512)]
TBMAX = 2048

_compiled_nc = None


def build_nc():
    import concourse.tile as tile
    from concourse import bacc, mybir

    f32 = mybir.dt.float32
    f16 = mybir.dt.float16
    AF = mybir.ActivationFunctionType
    ALU = mybir.AluOpType
    AX = mybir.AxisListType

    # Bacc (not plain Bass): its finalize() runs the TRN2 sync-wait
    # legalization (move_matmul_waits_to_ldweights / event-semaphore
    # splitting) that walrus codegen requires.
    nc = bacc.Bacc("TRN2", target_bir_lowering=False)
    x_d = nc.declare_dram_parameter("x", [D, T], f16, isOutput=False)
    w1p_d = nc.declare_dram_parameter("w1p", [128, NDT * DH], f16,
                                      isOutput=False)
    w2t_d = nc.declare_dram_parameter("w2t", [DH, D], f16, isOutput=False)
    b2p_d = nc.declare_dram_parameter("b2p", [128, NDT], f32, isOutput=False)
    sclb_d = nc.declare_dram_parameter("sclb", [DH, TC + 1], f32,
                                       isOutput=False)
    out_d = nc.declare_dram_parameter("out", [D, T], f16, isOutput=True)

    with tile.TileContext(nc) as tc:
        with (
            tc.tile_pool(name="xres", bufs=1) as xres,
            tc.tile_pool(name="small", bufs=1) as small,
            tc.tile_pool(name="ups", bufs=2) as ups,
            tc.tile_pool(name="psum_y", bufs=1, space="PSUM") as psum_y,
            tc.tile_pool(name="psum_g", bufs=4, space="PSUM") as psum_g,
        ):
            # x resident in SBUF: [128, 4, 8192] fp16 = 8 MB
            xt = xres.tile([128, NDT, T], f16, tag="x", name="x")
            w1s = small.tile([128, NDT, DH], f16, tag="w1")
            w2s = small.tile([DH, D], f16, tag="w2")
            b2s = small.tile([128, NDT], f32, tag="b2")
            sclb = small.tile([DH, TC + 1], f32, tag="sclb")
            b1s = sclb[:, 0:1]
            scl = sclb[:, 1:TC + 1]
            q = small.tile([DH, TC], f32, tag="q")      # per-chunk w1@x sums
            qs = small.tile([DH, TC], f32, tag="qs")    # causal prefix
            h32 = small.tile([DH, TC], f32, tag="h32")
            h16 = small.tile([DH, TC], f16, tag="h16")
            yp = [
                psum_y.tile([DH, SB], f32, tag=f"y{sb}", name=f"y{sb}")
                for sb in range(4)
            ]

            # Dummy 1-element sigmoid: forces the walrus-inserted
            # ACT_TABLE_LOAD for the sigmoid set to run during the startup
            # DMA window instead of stalling ACT before the first real
            # sigmoid mid-stream.
            dummy = small.tile([1, 2], f32, tag="dummy")
            nc.gpsimd.memset(dummy[:], 0.0)
            nc.scalar.activation(dummy[:, 1:2], dummy[:, 0:1], AF.Sigmoid)

            def load_block(eng, t0, TB):
                eng.dma_start(
                    xt[:, :, t0:t0 + TB],
                    x_d[:, t0:t0 + TB].rearrange("(k p) t -> p k t", p=128),
                )

            # SP ring (fast): the block-0 critical path first (tiny weights,
            # then the b0 x-block), then its share of the loads; ALL the
            # steady-state stores are issued on this ring later so the ACT
            # engine never spends mid-kernel time on DMA issue.
            nc.sync.dma_start(
                w1s[:], w1p_d[:].rearrange("p (d h) -> p d h", d=NDT)
            )
            nc.sync.dma_start(sclb[:], sclb_d[:])
            for bi in (0, 1, 3, 5):
                load_block(nc.sync, *TBLOCKS[bi])
            # ACT ring: the gate-path weights + two mid loads, all issued
            # up front before the first sigmoid.
            nc.scalar.dma_start(w2s[:], w2t_d[:])
            nc.scalar.dma_start(b2s[:], b2p_d[:])
            for bi in (2, 4):
                load_block(nc.scalar, *TBLOCKS[bi])

            # Causal pipeline: gate for chunk c needs only x[:, :16(c+1)].
            # Block k's gate multiplies + stores are emitted AFTER block
            # k+1's reduce/scan stage so the in-order DVE queue never stalls
            # waiting for the ACT sigmoid-upsample (software pipelining).
            deferred = None
            sbg = 0  # rotating PSUM bank assignment for Y sub-blocks

            def emit_mults(items, tail=False):
                for di, t0_, TB_, u_ in items:
                    xv = xt[:, di, t0_:t0_ + TB_]
                    nc.vector.tensor_tensor(xv, xv, u_[:, :TB_], op=ALU.mult)
                    if tail:
                        # tail: per-d-tile stores, alternating rings (the
                        # ACT ring is idle by now), right behind each
                        # multiply so the drain pipelines
                        deng = nc.sync if di < 2 else nc.scalar
                        deng.dma_start(
                            out_d[di * 128:(di + 1) * 128, t0_:t0_ + TB_], xv
                        )
                if not tail:
                    di, t0_, TB_, _ = items[0]
                    for half in range(2):
                        nc.sync.dma_start(
                            out_d[half * 256:(half + 1) * 256,
                                  t0_:t0_ + TB_].rearrange(
                                      "(k p) t -> p k t", p=128),
                            xt[:, 2 * half:2 * half + 2, t0_:t0_ + TB_],
                        )

            for tb, (t0, TB) in enumerate(TBLOCKS):
                CB = TB // CS
                c0 = t0 // CS
                nsb = TB // SB
                banks = [(sbg + i) % 4 for i in range(nsb)]
                sbg += nsb
                # Y = w1 @ x for this block: ki-outer so each stationary
                # w1-slice is reused across the sub-blocks while PSUM
                # accumulation groups interleave across banks.
                for ki in range(NDT):
                    for sb in range(nsb):
                        ts = t0 + sb * SB
                        nc.tensor.matmul(
                            yp[banks[sb]][:],
                            w1s[:, ki, :],
                            xt[:, ki, ts:ts + SB],
                            start=(ki == 0),
                            stop=(ki == NDT - 1),
                        )
                # chunk sums of Y: windowed reduce straight off PSUM
                for sb in range(nsb):
                    cc = c0 + sb * CSB
                    nc.vector.reduce_sum(
                        q[:, cc:cc + CSB],
                        yp[banks[sb]][:].rearrange("p (c j) -> p c j", j=CS),
                        axis=AX.X,
                    )
                # running causal prefix over this block (carry = last col)
                nc.vector.tensor_tensor_scan(
                    qs[:, c0:c0 + CB],
                    q[:, c0:c0 + CB],
                    q[:, c0:c0 + CB],
                    0.0 if tb == 0 else qs[:, c0 - 1:c0],
                    op0=ALU.add,
                    op1=ALU.bypass,
                )
                # h = relu(prefix * 1/(16(c+1)) + b1), relu on DVE so ACT
                # only ever needs the sigmoid table set
                nc.vector.tensor_mul(
                    h32[:, c0:c0 + CB], qs[:, c0:c0 + CB], scl[:, c0:c0 + CB]
                )
                nc.vector.tensor_scalar(
                    h16[:, c0:c0 + CB], h32[:, c0:c0 + CB],
                    b1s, 0.0, op0=ALU.add, op1=ALU.max,
                )
                last = tb == len(TBLOCKS) - 1
                if last and deferred is not None:
                    # flush the previous block's multiplies first so the
                    # tail drains in order
                    emit_mults(deferred)
                    deferred = None
                cur = []
                for di in range(NDT):
                    gp = psum_g.tile([128, TBMAX // CS], f32, tag="g",
                                     name="gp")
                    nc.tensor.matmul(
                        gp[:, :CB],
                        w2s[:, di * 128:(di + 1) * 128],
                        h16[:, c0:c0 + CB],
                        start=True,
                        stop=True,
                    )
                    # fused sigmoid + 16x upsample: broadcast-read the
                    # PSUM column per chunk, write the dense fp16 gate
                    u = ups.tile(
                        [128, TBMAX], f16, tag=f"u{di}", name=f"u{di}"
                    )
                    nc.scalar.activation(
                        u[:, :TB].rearrange("p (c j) -> p c j", j=CS),
                        gp[:, :CB].unsqueeze(2).broadcast_to([128, CB, CS]),
                        AF.Sigmoid,
                        bias=b2s[:, di:di + 1],
                    )
                    if last:
                        # tail block: multiply right behind each sigmoid so
                        # the drain pipelines at d-tile granularity
                        emit_mults([(di, t0, TB, u)], tail=True)
                    else:
                        cur.append((di, t0, TB, u))
                if deferred is not None:
                    emit_mults(deferred)
                deferred = cur if not last else None
    # run_bass_via_pjrt serializes nc.m as-is; Bacc defers register
    # allocation and TRN2 sync-wait legalization to finalize(), so it must
    # run here or walrus rejects the BIR.
    nc.finalize()
    return nc


def _host_inputs(x, w1, b1, w2, b2, chunksize):
    x = np.asarray(x)
    w1 = np.asarray(w1, dtype=np.float32)
    b1 = np.ascontiguousarray(np.asarray(b1, dtype=np.float32))
    w2 = np.asarray(w2, dtype=np.float32)
    b2 = np.asarray(b2, dtype=np.float32)
    cs = int(chunksize)
    assert cs == CS and x.shape == (B, D, T), (cs, x.shape)
    x16 = np.ascontiguousarray(x.astype(np.float16))
    # w1 pre-swizzled partition-major: w1p[p, k*DH+h] = w1[h, k*128+p]
    w1p = np.ascontiguousarray(
        w1.T.astype(np.float16).reshape(NDT, 128, DH)
        .transpose(1, 0, 2).reshape(128, NDT * DH)
    )
    w2t = np.ascontiguousarray(w2.T.astype(np.float16))      # [DH, D]
    b2p = np.ascontiguousarray(b2.reshape(NDT, 128).T)       # [128, NDT]
    scale = 1.0 / (CS * np.arange(1, TC + 1, dtype=np.float32))
    sclb = np.ascontiguousarray(np.concatenate(
        [np.broadcast_to(b1[:, None], (DH, 1)),
         np.broadcast_to(scale, (DH, TC))], axis=1,
    ))
    shared = dict(w1p=w1p, w2t=w2t, b2p=b2p, sclb=sclb)
    return x16, shared


def kernel(x, w1, b1, w2, b2, chunksize):
    global _compiled_nc
    from concourse.bass_utils import run_bass_kernel_spmd

    x16, shared = _host_inputs(x, w1, b1, w2, b2, chunksize)
    if _compiled_nc is None:
        _compiled_nc = build_nc()
    in_maps = [
        {"x": np.ascontiguousarray(x16[i]), **shared} for i in range(NCORES)
    ]
    res = run_bass_kernel_spmd(_compiled_nc, in_maps, list(range(NCORES)))
    out = np.stack(
        [res.results[i]["out"] for i in range(NCORES)], axis=0
    ).astype(np.float32)
    return out


# revision 19
# speedup vs baseline: 1.6750x; 1.0193x over previous
"""Trainium2 Bass kernel for nn_CausalSE: causal cumulative-average pooling
+ squeeze-excite gating, data-parallel over batch (one NeuronCore per batch
element).

Reference math per batch element (D=512, T=8192, chunk=16, Tc=512):
    avg    = cumsum(x, t) / (t+1)
    pooled = avg[:, 15::16]                          # [D, Tc]
    h      = relu(w1 @ pooled + b1)                  # [64, Tc]
    g      = sigmoid(w2 @ h + b2)                    # [D, Tc]
    out    = repeat(g, 16, t)[:, :T] * x

The kernel is HBM-bound: per core it streams x in and out once.  x
crosses HBM as fp16 (host converts both ways): ~16.8 MB at the ~400
GB/s per-core aggregate DMA rate => ~42us floor + ~10us of fixed NEFF
preamble/postamble, so every compute engine must stay under ~40us and
the per-block serial chain (load -> w1-matmul -> scan -> gate-matmul
-> sigmoid -> multiply -> store) must pipeline across blocks.

Structure (v1 was DVE-bound at ~55us busy; failed experiments: PSUM
same-address broadcast-accumulate output APs lose updates (RMW
hazard), strided moving-operand matmuls run ~5x slow):
  - Chunk sums ride the (otherwise idle) PE: w1 @ chunk_sum(x) ==
    chunk_sum(w1 @ x), so the PE computes Y = w1 @ x (4 ki
    accumulation steps into PSUM [64, 512] banks) and DVE
    windowed-reduces the 8x-smaller Y.
  - relu rides DVE as tensor_scalar(add b1, max 0); b2 is folded into
    the gate matmul (h gets a constant 1.0 row 64, w2 a b2 row), so
    the 4 per-block sigmoid+16x-upsample ACTIVATEs merge into ONE
    (saves the 352-cycle ACT fixed cost 18x and all ACT bias reads)
    and ACT only ever needs the sigmoid table set (a dummy 1-element
    sigmoid up front pulls the table load into the startup window).
  - Two-level software pipelining: the gate matmuls + sigmoid of
    block k are emitted after block k+1's Y-matmuls (PE runs the
    Y-stream dense, keeping the HAM clock-gate warm), and the gate
    multiplies + stores of block k are emitted two iterations behind
    (DVE never waits on ACT).
  - DMA: one load per t-block carrying all 4 d-tiles; block 0 is
    small (256 cols) and goes FIRST on the SP ring so compute ramps
    at ~10us; w1 is host-pre-swizzled partition-major (the naive
    (d p)->p d rearrange makes 128-byte descriptors that crawl); b1
    rides column 0 of the scale tensor.  Stores pair d-tiles: d01 on
    the SP ring, d23 on the ACT ring.  GpSimd issues nothing.
"""

import sys

for _p in ("/opt/trn_rl_repo",):
    if _p not in sys.path:
        sys.path.insert(0, _p)

import numpy as np

B, D, T = 8, 512, 8192
DH = 64          # bottleneck dim = D // 8
CS = 16          # chunksize
TC = T // CS     # 512 chunks
NCORES = 8
NDT = D // 128   # 4 partition tiles of x / out
SB = 512         # max Y-matmul sub-block (one PSUM bank of fp32)
TBLOCKS = [(0, 256), (256, 1280), (1536, 2048), (3584, 2048),
           (5632, 1792), (7424, 768)]
TBMAX = 2048

_compiled_nc = None


def build_nc():
    import concourse.tile as tile
    from concourse import bacc, mybir

    f32 = mybir.dt.float32
    f16 = mybir.dt.float16
    AF = mybir.ActivationFunctionType
    ALU = mybir.AluOpType
    AX = mybir.AxisListType

    # Bacc (not plain Bass): its finalize() runs the TRN2 sync-wait
    # legalization (move_matmul_waits_to_ldweights / event-semaphore
    # splitting) that walrus codegen requires.
    nc = bacc.Bacc("TRN2", target_bir_lowering=False)
    x_d = nc.declare_dram_parameter("x", [D, T], f16, isOutput=False)
    w1p_d = nc.declare_dram_parameter("w1p", [128, NDT * DH], f16,
                                      isOutput=False)
    w2e_d = nc.declare_dram_parameter("w2e", [DH + 1, D], f16, isOutput=False)
    sclb_d = nc.declare_dram_parameter("sclb", [DH, TC + 1], f32,
                                       isOutput=False)
    out_d = nc.declare_dram_parameter("out", [D, T], f16, isOutput=True)

    with tile.TileContext(nc) as tc:
        with (
            tc.tile_pool(name="xres", bufs=1) as xres,
            tc.tile_pool(name="small", bufs=1) as small,
            tc.tile_pool(name="ups", bufs=3) as ups,
            tc.tile_pool(name="psum_y", bufs=1, space="PSUM") as psum_y,
            tc.tile_pool(name="psum_g", bufs=2, space="PSUM") as psum_g,
        ):
            # x resident in SBUF: [128, 4, 8192] fp16 = 8 MB
            xt = xres.tile([128, NDT, T], f16, tag="x", name="x")
            w1s = small.tile([128, NDT, DH], f16, tag="w1")
            w2s = small.tile([DH + 1, D], f16, tag="w2")
            sclb = small.tile([DH, TC + 1], f32, tag="sclb")
            b1s = sclb[:, 0:1]
            scl = sclb[:, 1:TC + 1]
            q = small.tile([DH, TC], f32, tag="q")      # per-chunk w1@x sums
            qs = small.tile([DH, TC], f32, tag="qs")    # causal prefix
            h32 = small.tile([DH, TC], f32, tag="h32")
            # h with a constant 1.0 row DH that turns the gate matmul's
            # extra w2-row (= b2) into the bias add
            h16 = small.tile([DH + 1, TC], f16, tag="h16")
            yp = [
                psum_y.tile([DH, SB], f32, tag=f"y{sb}", name=f"y{sb}")
                for sb in range(4)
            ]

            nc.vector.memset(h16[DH:DH + 1, :], 1.0)

            # Dummy 1-element sigmoid: forces the walrus-inserted
            # ACT_TABLE_LOAD for the sigmoid set to run during the startup
            # DMA window instead of stalling ACT before the first real
            # sigmoid mid-stream.
            dummy = small.tile([1, 2], f32, tag="dummy")
            nc.gpsimd.memset(dummy[:], 0.0)
            nc.scalar.activation(dummy[:, 1:2], dummy[:, 0:1], AF.Sigmoid)

            def load_block(eng, t0, TB):
                eng.dma_start(
                    xt[:, :, t0:t0 + TB],
                    x_d[:, t0:t0 + TB].rearrange("(k p) t -> p k t", p=128),
                )

            # SP ring (fast): block 0 first, then the weights its chain
            # needs, then its share of the loads; d01 stores follow later.
            load_block(nc.sync, *TBLOCKS[0])
            nc.sync.dma_start(
                w1s[:], w1p_d[:].rearrange("p (d h) -> p d h", d=NDT)
            )
            nc.sync.dma_start(sclb[:], sclb_d[:])
            for bi in (1, 3, 5):
                load_block(nc.sync, *TBLOCKS[bi])
            # ACT ring: the gate weights + two mid loads, all issued up
            # front before the first sigmoid; d23 stores follow later.
            nc.scalar.dma_start(w2s[:], w2e_d[:])
            for bi in (2, 4):
                load_block(nc.scalar, *TBLOCKS[bi])

            sbg = 0  # rotating PSUM bank assignment for Y sub-blocks

            def prefix_stage(tb):
                """Y-matmuls + reduces + scan + scale + bias-relu."""
                nonlocal sbg
                t0, TB = TBLOCKS[tb]
                CB = TB // CS
                c0 = t0 // CS
                subs = [SB] * (TB // SB) + ([TB % SB] if TB % SB else [])
                banks = [(sbg + i) % 4 for i in range(len(subs))]
                sbg += len(subs)
                for ki in range(NDT):
                    ts = t0
                    for sb, w in enumerate(subs):
                        nc.tensor.matmul(
                            yp[banks[sb]][:, :w],
                            w1s[:, ki, :],
                            xt[:, ki, ts:ts + w],
                            start=(ki == 0),
                            stop=(ki == NDT - 1),
                        )
                        ts += w
                cc = c0
                for sb, w in enumerate(subs):
                    nc.vector.reduce_sum(
                        q[:, cc:cc + w // CS],
                        yp[banks[sb]][:, :w].rearrange(
                            "p (c j) -> p c j", j=CS),
                        axis=AX.X,
                    )
                    cc += w // CS
                nc.vector.tensor_tensor_scan(
                    qs[:, c0:c0 + CB],
                    q[:, c0:c0 + CB],
                    q[:, c0:c0 + CB],
                    0.0 if tb == 0 else qs[:, c0 - 1:c0],
                    op0=ALU.add,
                    op1=ALU.bypass,
                )
                nc.vector.tensor_mul(
                    h32[:, c0:c0 + CB], qs[:, c0:c0 + CB], scl[:, c0:c0 + CB]
                )
                nc.vector.tensor_scalar(
                    h16[:DH, c0:c0 + CB], h32[:, c0:c0 + CB],
                    b1s, 0.0, op0=ALU.add, op1=ALU.max,
                )

            def gate_stage(tb):
                """4 gate matmuls (bias via the 1.0 h-row) + ONE merged
                sigmoid + 16x upsample ACTIVATE for all 4 d-tiles."""
                t0, TB = TBLOCKS[tb]
                CB = TB // CS
                c0 = t0 // CS
                gp = psum_g.tile([128, NDT, TBMAX // CS], f32, tag="g",
                                 name="gp")
                for di in range(NDT):
                    nc.tensor.matmul(
                        gp[:, di, :CB],
                        w2s[:, di * 128:(di + 1) * 128],
                        h16[:, c0:c0 + CB],
                        start=True,
                        stop=True,
                    )
                u = ups.tile([128, NDT, TBMAX], f16, tag="u", name="u")
                nc.scalar.activation(
                    u[:, :, :TB].rearrange("p k (c j) -> p k c j", j=CS),
                    gp[:, :, :CB].unsqueeze(3).broadcast_to(
                        [128, NDT, CB, CS]),
                    AF.Sigmoid,
                )
                return u

            def mult_stage(tb, u, tail=False):
                """Gate multiplies + stores (d01 -> SP ring, d23 -> ACT)."""
                t0, TB = TBLOCKS[tb]
                for di in range(NDT):
                    xv = xt[:, di, t0:t0 + TB]
                    nc.vector.tensor_tensor(
                        xv, xv, u[:, di, :TB], op=ALU.mult
                    )
                    if tail:
                        deng = nc.sync if di < 2 else nc.scalar
                        deng.dma_start(
                            out_d[di * 128:(di + 1) * 128, t0:t0 + TB], xv
                        )
                    elif di % 2 == 1:
                        half = di // 2
                        deng = nc.sync if half == 0 else nc.scalar
                        deng.dma_start(
                            out_d[half * 256:(half + 1) * 256,
                                  t0:t0 + TB].rearrange(
                                      "(k p) t -> p k t", p=128),
                            xt[:, 2 * half:2 * half + 2, t0:t0 + TB],
                        )

            NB = len(TBLOCKS)
            ulist = {}
            for tb in range(NB):
                prefix_stage(tb)
                if tb >= 1:
                    ulist[tb - 1] = gate_stage(tb - 1)
                if tb >= 2:
                    mult_stage(tb - 2, ulist.pop(tb - 2))
            ulist[NB - 1] = gate_stage(NB - 1)
            mult_stage(NB - 2, ulist.pop(NB - 2))
            mult_stage(NB - 1, ulist.pop(NB - 1), tail=True)
    # run_bass_via_pjrt serializes nc.m as-is; Bacc defers register
    # allocation and TRN2 sync-wait legalization to finalize(), so it must
    # run here or walrus rejects the BIR.
    nc.finalize()
    return nc


def _host_inputs(x, w1, b1, w2, b2, chunksize):
    x = np.asarray(x)
    w1 = np.asarray(w1, dtype=np.float32)
    b1 = np.ascontiguousarray(np.asarray(b1, dtype=np.float32))
    w2 = np.asarray(w2, dtype=np.float32)
    b2 = np.asarray(b2, dtype=np.float32)
    cs = int(chunksize)
    assert cs == CS and x.shape == (B, D, T), (cs, x.shape)
    x16 = np.ascontiguousarray(x.astype(np.float16))
    # w1 pre-swizzled partition-major: w1p[p, k*DH+h] = w1[h, k*128+p]
    w1p = np.ascontiguousarray(
        w1.T.astype(np.float16).reshape(NDT, 128, DH)
        .transpose(1, 0, 2).reshape(128, NDT * DH)
    )
    # w2 transposed with b2 as the extra row DH (paired with h's 1.0 row)
    w2e = np.ascontiguousarray(np.concatenate(
        [w2.T, b2[None, :]], axis=0).astype(np.float16))     # [DH+1, D]
    scale = 1.0 / (CS * np.arange(1, TC + 1, dtype=np.float32))
    sclb = np.ascontiguousarray(np.concatenate(
        [np.broadcast_to(b1[:, None], (DH, 1)),
         np.broadcast_to(scale, (DH, TC))], axis=1,
    ))
    shared = dict(w1p=w1p, w2e=w2e, sclb=sclb)
    return x16, shared


def kernel(x, w1, b1, w2, b2, chunksize):
    global _compiled_nc
    from concourse.bass_utils import run_bass_kernel_spmd

    x16, shared = _host_inputs(x, w1, b1, w2, b2, chunksize)
    if _compiled_nc is None:
        _compiled_nc = build_nc()
    in_maps = [
        {"x": np.ascontiguousarray(x16[i]), **shared} for i in range(NCORES)
    ]
    res = run_bass_kernel_spmd(_compiled_nc, in_maps, list(range(NCORES)))
    out = np.stack(
        [res.results[i]["out"] for i in range(NCORES)], axis=0
    ).astype(np.float32)
    return out
